# revision 1
# baseline (speedup 1.0000x reference)
"""Trainium2 Bass kernel for nn_GAT_Decoder (one decode step of a GAT decoder).

Strategy (per spec sharding hint): pure data parallel, batch sharded 8 ways
(32 batch elements per core), weights replicated.

The three O(B*N*D^2) projections (K, V, Kp) of the reference are eliminated
algebraically; only O(B*N*D) work streams through the PE:
  compat[b,h,n] = qhat[b,h,:] . E[b,n,:]   with qhat = state @ Wq_h @ Wk_h^T
  ctx[b,h,:]    = attn[b,h,:] @ E[b]                 (contract n first)
  o[b,C_h]      = ctx[b,h,:] @ Wv[:,C_h]
  phat          = o @ G^T,  G = Wk_ptr @ Wo^T        (weights folded on device)
  compat2[b,n]  = phat[b,:] . E[b,n,:]
Masked rows (~50%) are compacted out host-side (gather); masked/pad scores are
exactly 0, matching the reference's softmax(-inf) = 0.

Device layout: per subgroup of 4 batches, compat/compat2/ctx outputs are packed
at partition offsets {0,32,64,96} via tile_position, so softmax/tanh run as
full-128-partition vector ops. E^T is materialized per batch by PE transposes
(or DMA'd from a host-transposed copy when HOST_ET).
"""

import numpy as np

B, N, D, H = 256, 1000, 512, 8
HD = D // H
NCORES = 8
BPC = B // NCORES          # batches per core
SG = 4                     # batches per subgroup (partition packing factor)
ND = D // 128              # 4 contraction chunks

NORM_MHA = float(1.0 / np.sqrt(HD))
NORM_PTR = float(1.0 / np.sqrt(D))
MASKVAL = 1e30

# knobs (overridable for experiments)
HOST_ET = False            # True: DMA host-transposed E^T instead of PE transposes
USE_F32R = True            # float32r for the big E streams
TR_F32R = False            # float32r transposes (exactness verified by probe)


def _build(n_c, bpc, host_et, use_f32r, tr_f32r):
    from concourse import bacc
    import concourse.mybir as mybir
    import concourse.tile as tile
    from concourse.masks import make_identity

    dt = mybir.dt
    AF = mybir.ActivationFunctionType
    ALU = mybir.AluOpType
    nt = n_c // 128
    nh = n_c // 2              # free size of each matmul half (<=512)
    nsg = bpc // SG
    f32 = dt.float32
    if use_f32r == "bf16":
        sdt = dt.bfloat16
    elif use_f32r:
        sdt = dt.float32r
    else:
        sdt = f32

    pdt = dt.float32r if use_f32r else f32   # phase-0 dtype
    nc = bacc.Bacc("TRN2", target_bir_lowering=False, debug=False)

    # ---- DRAM I/O ----
    Ec_d = nc.dram_tensor("Ec", [bpc, n_c, D], sdt, kind="ExternalInput")
    n_et = bpc if host_et is True else (bpc // 2 if host_et == "hybrid" else 0)
    ET_d = (nc.dram_tensor("EcT", [n_et, D, n_c], sdt, kind="ExternalInput")
            if n_et else None)
    mf8_d = nc.dram_tensor("maskf8", [bpc, 8, n_c], f32, kind="ExternalInput")
    mfp_d = nc.dram_tensor("maskfp", [nsg, 128, n_c], f32, kind="ExternalInput")
    wfc_d = nc.dram_tensor("W_fc", [D + 1, D], pdt, kind="ExternalInput")
    wfc1_d = nc.dram_tensor("W_fc1", [D, D], pdt, kind="ExternalInput")
    wq_d = nc.dram_tensor("Wq", [D, D], pdt, kind="ExternalInput")
    wkT_d = nc.dram_tensor("WkT", [D, D], pdt, kind="ExternalInput")
    wv_d = nc.dram_tensor("Wv", [D, D], sdt, kind="ExternalInput")
    woT_d = nc.dram_tensor("WoT", [D, D], pdt, kind="ExternalInput")
    wpT_d = nc.dram_tensor("WpT", [D, D], pdt, kind="ExternalInput")
    wlastT_d = nc.dram_tensor("wlastT", [128, ND], f32, kind="ExternalInput")
    e0T_d = nc.dram_tensor("E0T", [D, bpc], pdt, kind="ExternalInput")
    poolT_d = nc.dram_tensor("poolT", [D, bpc], pdt, kind="ExternalInput")
    dcrep_d = nc.dram_tensor("dcrep", [128, bpc], f32, kind="ExternalInput")
    out_d = nc.dram_tensor("scores", [bpc, n_c], f32, kind="ExternalOutput")

    def w_ap(d):  # [512,512] dram -> [128, 4, 512]
        return d[0:D, :].rearrange("(c p) d -> p c d", p=128)

    def tpos(row, col):
        return None if (row == 0 and col == 0) else (row, col)

    with tile.TileContext(nc) as tc:
        with tc.tile_pool(name="const", bufs=1) as constp, \
             tc.tile_pool(name="wmain", bufs=1) as wmain, \
             tc.tile_pool(name="small", bufs=1) as smallp:
            ident = constp.tile([128, 128], f32, tag="ident")
            make_identity(nc, ident[:])
            if use_f32r:
                identr = constp.tile([128, 128], sdt, tag="identr")
                nc.vector.tensor_copy(identr[:], ident[:])
            else:
                identr = ident

            # persistent
            wv_t = wmain.tile([128, ND, D], sdt, tag="wv")
            gt_t = wmain.tile([128, ND, D], sdt, tag="gt")
            qhatT = wmain.tile([128, ND, bpc, H], sdt, tag="qhatT")
            nc.gpsimd.dma_start(wv_t[:], w_ap(wv_d))

            # ---------- phase 0 ----------
            with tc.tile_pool(name="w0", bufs=1) as w0, \
                 tc.tile_pool(name="ps0", bufs=2, space="PSUM") as ps0:
                wfc_t = w0.tile([128, ND, D], pdt, tag="wfc")
                wfc1_t = w0.tile([128, ND, D], pdt, tag="wfc1")
                wq_t = w0.tile([128, ND, D], pdt, tag="wq")
                wkT_t = w0.tile([128, ND, D], pdt, tag="wkT")
                woT_t = w0.tile([128, ND, D], pdt, tag="woT")
                wpT_t = w0.tile([128, ND, D], pdt, tag="wpT")
                wlast_t = w0.tile([128, ND], f32, tag="wlast")
                e0T_t = w0.tile([128, ND, bpc], pdt, tag="e0T")
                poolT_t = w0.tile([128, ND, bpc], pdt, tag="poolT")
                dcrep_t = w0.tile([128, bpc], f32, tag="dcrep")
                stateT = w0.tile([128, ND, bpc], pdt, tag="stateT")
                qT_t = w0.tile([128, ND, bpc], pdt, tag="qT")
                nc.gpsimd.dma_start(wfc_t[:], w_ap(wfc_d))
                nc.gpsimd.dma_start(wfc1_t[:], w_ap(wfc1_d))
                nc.gpsimd.dma_start(wq_t[:], w_ap(wq_d))
                nc.gpsimd.dma_start(wkT_t[:], w_ap(wkT_d))
                nc.gpsimd.dma_start(woT_t[:], w_ap(woT_d))
                nc.gpsimd.dma_start(wpT_t[:], w_ap(wpT_d))
                nc.gpsimd.dma_start(wlast_t[:], wlastT_d[:])
                nc.gpsimd.dma_start(e0T_t[:], e0T_d[:].rearrange("(c p) b -> p c b", p=128))
                nc.gpsimd.dma_start(poolT_t[:], poolT_d[:].rearrange("(c p) b -> p c b", p=128))
                nc.gpsimd.dma_start(dcrep_t[:], dcrep_d[:])

                # G^T = WoT.T @ WpT, scaled by norm_ptr
                for c in range(ND):
                    g_ps = ps0.tile([128, D], f32, tag="g_ps")
                    for kc in range(ND):
                        nc.tensor.matmul(g_ps[:], woT_t[:, kc, 128 * c:128 * (c + 1)],
                                         wpT_t[:, kc, :], start=(kc == 0), stop=(kc == ND - 1))
                    nc.scalar.mul(gt_t[:, c, :], g_ps[:], NORM_PTR)

                # stateT
                for c in range(ND):
                    st_ps = ps0.tile([128, bpc], f32, tag="st_ps")
                    for kc in range(ND):
                        nc.tensor.matmul(st_ps[:], wfc_t[:, kc, 128 * c:128 * (c + 1)],
                                         e0T_t[:, kc, :], start=(kc == 0), stop=False)
                    for kc in range(ND):
                        nc.tensor.matmul(st_ps[:], wfc1_t[:, kc, 128 * c:128 * (c + 1)],
                                         poolT_t[:, kc, :], start=False, stop=(kc == ND - 1))
                    nc.vector.scalar_tensor_tensor(
                        stateT[:, c, :], dcrep_t[:], wlast_t[:, c:c + 1], st_ps[:],
                        op0=ALU.mult, op1=ALU.add)

                # QT (scaled by norm_mha)
                for c in range(ND):
                    q_ps = ps0.tile([128, bpc], f32, tag="q_ps")
                    for kc in range(ND):
                        nc.tensor.matmul(q_ps[:], wq_t[:, kc, 128 * c:128 * (c + 1)],
                                         stateT[:, kc, :], start=(kc == 0), stop=(kc == ND - 1))
                    nc.scalar.mul(qT_t[:, c, :], q_ps[:], NORM_MHA)

                # qhatT
                for h in range(H):
                    pb = 64 * (h % 2)
                    for c in range(ND):
                        qq = ps0.tile([128, bpc], f32, tag="qq")
                        nc.tensor.matmul(
                            qq[:], wkT_t[pb:pb + 64, h // 2, 128 * c:128 * (c + 1)],
                            qT_t[pb:pb + 64, h // 2, :], start=True, stop=True)
                        nc.vector.tensor_copy(qhatT[:, c, :, h], qq[:])

            # ---------- main loop ----------
            # Software-pipelined: stage A(b) = load/transpose/compat/softmax,
            # stage B(b) = expT/ctx/o/oT (needs A(b)'s softmax). B(b) is
            # emitted after A(b+1) so PE never waits on the softmax chain.
            # Per subgroup s, phatT + compat2 + pointer stage C(s) follows
            # B(last batch of s). All f32r matmuls write PSUM at partition 0;
            # strips are [8,n]/[1,n] (free-dim bound, lane count irrelevant).
            with tc.tile_pool(name="epool", bufs=4) as epool, \
                 tc.tile_pool(name="etpool", bufs=8) as etpool, \
                 tc.tile_pool(name="mfpool", bufs=3) as mfpool, \
                 tc.tile_pool(name="mfppool", bufs=2) as mfppool, \
                 tc.tile_pool(name="smpool", bufs=3) as smpool, \
                 tc.tile_pool(name="stg", bufs=2) as stg, \
                 tc.tile_pool(name="xtpool", bufs=1) as xtpool, \
                 tc.tile_pool(name="tp_ps", bufs=3, space="PSUM") as tp_ps, \
                 tc.tile_pool(name="big_ps", bufs=3, space="PSUM") as big_ps, \
                 tc.tile_pool(name="cx_ps", bufs=2, space="PSUM") as cx_ps:
                st = {}
                sgst = {}

                def emit_A1(bc):
                    sg = bc // SG
                    if bc % SG == 0:
                        mfp_t = mfppool.tile([128, n_c], f32, tag="mfp")
                        nc.sync.dma_start(mfp_t[:], mfp_d[sg])
                        oT_sb = smpool.tile([128, ND, SG], sdt, tag="oT_sb")
                        sgst[sg] = dict(mfp=mfp_t, oT=oT_sb, et={})
                    e_t = epool.tile([128, nt, D], sdt, tag="E")
                    nc.sync.dma_start(
                        e_t[:], Ec_d[bc].rearrange("(t p) d -> p t d", p=128))
                    et_t = etpool.tile([128, ND, n_c], sdt, tag="ET")
                    use_dma_et = (host_et is True) or (host_et == "hybrid" and bc % 2 == 1)
                    if use_dma_et:
                        ei = bc if host_et is True else bc // 2
                        nc.sync.dma_start(
                            et_t[:], ET_d[ei].rearrange("(c p) n -> p c n", p=128))
                    else:
                        for t in range(nt):
                            tp = tp_ps.tile([128, D], sdt, tag="tp")
                            for c in range(ND):
                                nc.tensor.transpose(
                                    tp[:, 128 * c:128 * (c + 1)],
                                    e_t[:, t, 128 * c:128 * (c + 1)],
                                    identr[:])
                            eng = nc.vector.tensor_copy if t % 2 == 0 else nc.scalar.copy
                            eng(et_t[:, :, 128 * t:128 * (t + 1)],
                                tp[:, :].rearrange("p (c x) -> p c x", c=ND))
                    sgst[sg]['et'][bc] = et_t
                    mf8_t = mfpool.tile([8, n_c], f32, tag="mf8")
                    nc.sync.dma_start(mf8_t[:], mf8_d[bc])
                    st[bc] = dict(e=e_t, et=et_t, mf8=mf8_t)

                def emit_A2(bc):
                    et_t, mf8_t = st[bc]['et'], st[bc]['mf8']
                    cp0 = big_ps.tile([8, nh], f32, tag="big")
                    cp1 = big_ps.tile([8, nh], f32, tag="big")
                    for half, cph in ((0, cp0), (1, cp1)):
                        for c in range(ND):
                            nc.tensor.matmul(
                                cph[:, :], qhatT[:, c, bc, :],
                                et_t[:, c, half * nh:(half + 1) * nh],
                                start=(c == 0), stop=(c == ND - 1))
                    u8 = stg.tile([8, n_c], f32, tag="u8")
                    nc.vector.tensor_sub(u8[:, 0:nh], cp0[:, :], mf8_t[:, 0:nh])
                    nc.vector.tensor_sub(u8[:, nh:n_c], cp1[:, :], mf8_t[:, nh:n_c])
                    s8 = smallp.tile([8, 1], f32, tag="s8")
                    ex8 = stg.tile([8, n_c], sdt, tag="ex8")
                    nc.scalar.activation(ex8[:, :], u8[:, :], AF.Exp,
                                         bias=0.0, scale=1.0, accum_out=s8[:])
                    r8 = smallp.tile([8, 1], f32, tag="r8")
                    nc.vector.reciprocal(r8[:], s8[:])
                    st[bc].update(ex8=ex8, r8=r8)

                def emit_expT(bc):
                    ex8 = st[bc]['ex8']
                    tpT = tp_ps.tile([128, D], sdt, tag="tp")
                    for t in range(nt):
                        nc.tensor.transpose(
                            tpT[:, 8 * t:8 * (t + 1)],
                            ex8[:, 128 * t:128 * (t + 1)], identr[0:8, 0:8])
                    expT_t = smpool.tile([128, nt, 8], sdt, tag="expT")
                    nc.vector.tensor_copy(
                        expT_t[:, :, :],
                        tpT[:, 0:8 * nt].rearrange("p (t x) -> p t x", t=nt))
                    st[bc]['expT'] = expT_t

                def emit_ctxu(bc):
                    # ctx = (unnormalized expT).T @ E, normalization folded into
                    # the PSUM->SBUF copy via scale=1/sum
                    expT_t, e_t, r8 = st[bc]['expT'], st[bc]['e'], st[bc]['r8']
                    cxp = cx_ps.tile([8, D], f32, tag="cx")
                    for t in range(nt):
                        nc.tensor.matmul(
                            cxp[:, :], expT_t[:, t, :], e_t[:, t, :],
                            start=(t == 0), stop=(t == nt - 1))
                    ctx8 = stg.tile([8, D], sdt, tag="ctx8")
                    nc.scalar.activation(ctx8[:, :], cxp[:, :], AF.Copy,
                                         bias=0.0, scale=r8[:])
                    st[bc]['ctx8'] = ctx8

                def emit_ctxT(bc):
                    ctx8 = st[bc]['ctx8']
                    tpC = tp_ps.tile([128, D], sdt, tag="tp")
                    for c in range(ND):
                        nc.tensor.transpose(
                            tpC[:, 8 * c:8 * (c + 1)],
                            ctx8[:, 128 * c:128 * (c + 1)], identr[0:8, 0:8])
                    ctxT_t = smpool.tile([128, ND, 8], sdt, tag="ctxT")
                    nc.vector.tensor_copy(
                        ctxT_t[:, :, :],
                        tpC[:, 0:8 * ND].rearrange("p (c x) -> p c x", c=ND))
                    st[bc]['ctxT'] = ctxT_t

                def emit_o(bc):
                    ctxT_t = st[bc]['ctxT']
                    op = cx_ps.tile([8, D], f32, tag="cx")
                    for c in range(ND):
                        nc.tensor.matmul(
                            op[:, :], ctxT_t[:, c, :], wv_t[:, c, :],
                            start=(c == 0), stop=(c == ND - 1))
                    o8 = stg.tile([8, D], sdt, tag="o8")
                    nc.scalar.copy(o8[:, :], op[:, :])
                    st[bc]['o8'] = o8

                def emit_oT(bc):
                    sg, j = bc // SG, bc % SG
                    o8 = st[bc]['o8']
                    tpO = tp_ps.tile([128, D], sdt, tag="tp")
                    for c in range(ND):
                        nc.tensor.transpose(
                            tpO[:, 8 * c:8 * (c + 1)],
                            o8[:, 128 * c:128 * (c + 1)], identr[0:8, 0:8])
                    oT_f = sgst[sg]['oT'][:, :, :].rearrange("p c j -> p (c j)")
                    for c in range(ND):
                        fi = c * SG + j
                        nc.vector.tensor_copy(
                            oT_f[0:64, fi:fi + 1], tpO[0:64, 10 * c:10 * c + 1])
                        nc.vector.tensor_copy(
                            oT_f[64:128, fi:fi + 1], tpO[64:128, 10 * c + 1:10 * c + 2])
                    del st[bc]

                def emit_C1(sg):
                    oT_sb = sgst[sg]['oT']
                    pp_ps = tp_ps.tile([128, 4 * SG], f32, tag="tp")
                    for c2 in range(ND):
                        for c in range(ND):
                            nc.tensor.matmul(
                                pp_ps[:, SG * c2:SG * (c2 + 1)],
                                gt_t[:, c, 128 * c2:128 * (c2 + 1)], oT_sb[:, c, :],
                                start=(c == 0), stop=(c == ND - 1))
                    phatT_t = smpool.tile([128, ND, SG], sdt, tag="phatT")
                    nc.vector.tensor_copy(
                        phatT_t[:, :, :],
                        pp_ps[:, :].rearrange("p (c j) -> p c j", c=ND))
                    sgst[sg]['phatT'] = phatT_t

                def emit_C2(sg):
                    phatT_t, mfp_t = sgst[sg]['phatT'], sgst[sg]['mfp']
                    ptr_pk = xtpool.tile([128, n_c], f32, tag="ptr")
                    for j in range(SG):
                        et_t = sgst[sg]['et'][SG * sg + j]
                        q0 = big_ps.tile([1, nh], f32, tag="big")
                        q1 = big_ps.tile([1, nh], f32, tag="big")
                        for half, qh in ((0, q0), (1, q1)):
                            for c in range(ND):
                                nc.tensor.matmul(
                                    qh[:, :], phatT_t[:, c, j:j + 1],
                                    et_t[:, c, half * nh:(half + 1) * nh],
                                    start=(c == 0), stop=(c == ND - 1))
                        p28 = stg.tile([1, n_c], f32, tag="p28")
                        nc.scalar.copy(p28[:, 0:nh], q0[:, :])
                        nc.scalar.copy(p28[:, nh:n_c], q1[:, :])
                        nc.sync.dma_start(ptr_pk[32 * j:32 * j + 1, :], p28[:, :])

                    tn_t = xtpool.tile([128, n_c], f32, tag="x1")
                    nc.scalar.activation(tn_t[:], ptr_pk[:, :], AF.Tanh)
                    x_t = xtpool.tile([128, n_c], f32, tag="x2")
                    nc.vector.scalar_tensor_tensor(
                        x_t[:, :], tn_t[:, :], 10.0, mfp_t[:, :],
                        op0=ALU.mult, op1=ALU.subtract)
                    e2_t = xtpool.tile([128, n_c], f32, tag="x1")
                    s2 = smallp.tile([128, 1], f32, tag="s2")
                    nc.scalar.activation(e2_t[:], x_t[:, :], AF.Exp,
                                         bias=0.0, scale=1.0, accum_out=s2[:])
                    r2 = smallp.tile([128, 1], f32, tag="r2")
                    nc.vector.reciprocal(r2[:], s2[:])
                    sc_t = xtpool.tile([128, n_c], f32, tag="x2")
                    nc.vector.tensor_scalar_mul(sc_t[:], e2_t[:], r2[:])
                    for j in range(SG):
                        nc.sync.dma_start(out_d[SG * sg + j], sc_t[32 * j:32 * j + 1, :])
                    del sgst[sg]

                # staggered rounds: lags hide every cross-engine latency
                for r in range(bpc + 5):
                    if r < bpc:
                        emit_A1(r)
                    if 0 <= r - 4 < bpc:
                        emit_oT(r - 4)
                        if (r - 4) % SG == SG - 1:
                            emit_C1((r - 4) // SG)
                    if 0 <= r - 1 < bpc:
                        emit_expT(r - 1)
                    if r < bpc:
                        emit_A2(r)
                    if 0 <= r - 4 < bpc and (r - 4) % SG == SG - 1:
                        emit_C2((r - 4) // SG)
                    if 0 <= r - 1 < bpc:
                        emit_ctxu(r - 1)
                    if 0 <= r - 2 < bpc:
                        emit_ctxT(r - 2)
                    if 0 <= r - 3 < bpc:
                        emit_o(r - 3)

    nc.finalize()
    return nc


def _host_prep(inputs, n_c=None):
    E = np.ascontiguousarray(inputs['encoder_inputs'], dtype=np.float32)
    mask = np.asarray(inputs['mask'])
    unm = (mask == 0)
    counts = unm.sum(axis=1)
    if n_c is None:
        n_c = max(512, int(np.ceil(counts.max() / 128) * 128))
    idx = np.zeros((B, n_c), dtype=np.int64)
    maskf = np.full((B, n_c), MASKVAL, dtype=np.float32)
    for b in range(B):
        ii = np.nonzero(unm[b])[0]
        k = min(len(ii), n_c)
        idx[b, :k] = ii[:k]
        maskf[b, :k] = 0.0
    Ec = np.take_along_axis(E, idx[:, :, None], axis=1)   # [B, n_c, D]
    return Ec, idx, counts, maskf, n_c


def _in_maps(inputs, Ec, maskf, n_c, bpc=BPC, host_et=False, bf16=False):
    nsg = bpc // SG
    Ec32 = Ec
    if bf16:
        import ml_dtypes
        Ec = Ec.astype(ml_dtypes.bfloat16)
    W_fc = np.asarray(inputs['W_fc'], dtype=np.float32)
    wlastT = np.ascontiguousarray(W_fc[D].reshape(ND, 128).T)        # [128, 4]
    wkT = np.ascontiguousarray(np.asarray(inputs['Wk_mha']).T)
    woT = np.ascontiguousarray(np.asarray(inputs['Wo']).T)
    wpT = np.ascontiguousarray(np.asarray(inputs['Wk_ptr']).T)
    pool = np.asarray(inputs['pool'], dtype=np.float32)
    dc = np.asarray(inputs['dynamic_capacity'], dtype=np.float32)
    # maskf expanded: [nsg, 128, n_c] per core (each batch row replicated x32)
    maps = []
    for i in range(NCORES):
        b0 = i * bpc
        mfe = np.repeat(maskf[b0:b0 + bpc], 32, axis=0).reshape(nsg, SG * 32, n_c)
        mf8 = np.repeat(maskf[b0:b0 + bpc], 8, axis=0).reshape(bpc, 8, n_c)
        m = {
            "Ec": np.ascontiguousarray(Ec[b0:b0 + bpc]),
            "maskf8": np.ascontiguousarray(mf8),
            "maskfp": np.ascontiguousarray(mfe),
            "W_fc": W_fc,
            "W_fc1": np.asarray(inputs['W_fc1'], dtype=np.float32),
            "Wq": np.asarray(inputs['Wq'], dtype=np.float32),
            "WkT": wkT,
            "Wv": (np.asarray(inputs['Wv'], dtype=np.float32).astype(__import__('ml_dtypes').bfloat16)
                   if bf16 else np.asarray(inputs['Wv'], dtype=np.float32)),
            "WoT": woT,
            "WpT": wpT,
            "wlastT": wlastT,
            "E0T": np.ascontiguousarray(Ec32[b0:b0 + bpc, 0, :].T),
            "poolT": np.ascontiguousarray(pool[b0:b0 + bpc].T),
            "dcrep": np.ascontiguousarray(np.broadcast_to(dc[b0:b0 + bpc, 0], (128, bpc))),
        }
        if host_et is True:
            m["EcT"] = np.ascontiguousarray(Ec[b0:b0 + bpc].transpose(0, 2, 1))
        elif host_et == "hybrid":
            m["EcT"] = np.ascontiguousarray(Ec[b0 + 1:b0 + bpc:2].transpose(0, 2, 1))
        maps.append(m)
    return maps


_cache = {}


def _get_nc(n_c, bpc, host_et, use_f32r, tr_f32r):
    key = (n_c, bpc, host_et, use_f32r, tr_f32r)
    if key not in _cache:
        _cache[key] = _build(n_c, bpc, host_et, use_f32r, tr_f32r)
    return _cache[key]


def run(inputs, trace=False, host_et=HOST_ET, use_f32r=USE_F32R, tr_f32r=TR_F32R):
    from concourse.bass_utils import run_bass_kernel_spmd
    Ec, idx, counts, maskf, n_c = _host_prep(inputs)
    nc = _get_nc(n_c, BPC, host_et, use_f32r, tr_f32r)
    maps = _in_maps(inputs, Ec, maskf, n_c, BPC, host_et, bf16=(use_f32r == "bf16"))
    res = run_bass_kernel_spmd(nc, maps, list(range(NCORES)), trace=trace)
    scores = np.zeros((B, N), dtype=np.float32)
    for i in range(NCORES):
        sc = res.results[i]["scores"]
        for j in range(BPC):
            b = i * BPC + j
            c = counts[b]
            scores[b, idx[b, :c]] = sc[j, :c]
    return scores, res


def kernel(**inputs) -> np.ndarray:
    scores, _ = run(inputs, trace=False)
    return scores



# revision 18
# speedup vs baseline: 1.6365x; 1.6365x over previous
"""Trainium2 Bass kernel for nn_GAT_Decoder (one decode step of a GAT decoder).

Strategy (per spec sharding hint): pure data parallel, batch sharded 8 ways
(32 batch elements per core), weights replicated.

v2 design notes vs the earlier kernel:
- E streams in bf16, and E^T is transposed on the HOST and DMA'd (bf16), so
  the PE does no [128,128] E^T transposes at all.  Total E DMA (E + E^T in
  bf16) equals the old f32 E alone.
- Masked rows are compacted out host-side; padding rows of E / padding cols
  of E^T are ZERO, so padded compat entries are exactly 0 and contribute
  exp(0)=1 to softmax sums.  The host passes npad = n_c - count per batch and
  the kernel subtracts it from each softmax denominator.  No mask tensors on
  device at all.
- Batches are processed in quads (4).  All small-M matmuls (compat M=8,
  ctx M=8, compat2 M=1) are col-tiled via tile_position=(0,32j) so the four
  batches' matmuls run concurrently on disjoint PE column groups.
- o = ctx @ Wv (per-head diag blocks) and phat = G^T @ oT contract against
  *fixed* weights, so they run once per quad with the weight as stationary
  and the quad's 4 batches stacked on the moving free dim.
- The pointer softmax (tanh/exp/normalize) is computed once per core on a
  packed [32, n_c] tile instead of per-batch rows.
"""

import numpy as np

B, N, D, H = 256, 1000, 512, 8
HD = D // H
NCORES = 8
BPC = B // NCORES          # batches per core
QUAD = 4                   # batches per quad (PE column-group packing)
ND = D // 128              # 4 contraction chunks

NORM_MHA = float(1.0 / np.sqrt(HD))
NORM_PTR = float(1.0 / np.sqrt(D))
DEBUG = False              # adds intermediate DRAM dumps (quad 0)


def _build(n_c, bpc):
    from concourse import bacc
    import concourse.mybir as mybir
    import concourse.tile as tile
    from concourse.masks import make_identity

    dt = mybir.dt
    AF = mybir.ActivationFunctionType
    ALU = mybir.AluOpType
    f32 = dt.float32
    bf16 = dt.bfloat16
    pdt = dt.float32r          # phase-0 dtype

    nt = n_c // 128
    nh = n_c // 2
    nquad = bpc // QUAD

    nc = bacc.Bacc("TRN2", target_bir_lowering=False, debug=False)

    # ---- DRAM I/O ----
    Ec_d = nc.dram_tensor("Ec", [bpc, n_c, D], bf16, kind="ExternalInput")
    EcT_d = nc.dram_tensor("EcT", [bpc, D, n_c], bf16, kind="ExternalInput")
    wfc_d = nc.dram_tensor("W_fc", [D + 1, D], pdt, kind="ExternalInput")
    wfc1_d = nc.dram_tensor("W_fc1", [D, D], pdt, kind="ExternalInput")
    wq_d = nc.dram_tensor("Wq", [D, D], pdt, kind="ExternalInput")
    wkT_d = nc.dram_tensor("WkT", [D, D], pdt, kind="ExternalInput")
    wv_d = nc.dram_tensor("Wv", [D, D], bf16, kind="ExternalInput")
    woT_d = nc.dram_tensor("WoT", [D, D], pdt, kind="ExternalInput")
    wpT_d = nc.dram_tensor("WpT", [D, D], pdt, kind="ExternalInput")
    wlastT_d = nc.dram_tensor("wlastT", [128, ND], f32, kind="ExternalInput")
    e0T_d = nc.dram_tensor("E0T", [D, bpc], pdt, kind="ExternalInput")
    poolT_d = nc.dram_tensor("poolT", [D, bpc], pdt, kind="ExternalInput")
    dcrep_d = nc.dram_tensor("dcrep", [128, bpc], f32, kind="ExternalInput")
    npadq_d = nc.dram_tensor("npadq", [nquad, 128, 1], f32, kind="ExternalInput")
    npad32_d = nc.dram_tensor("npad32", [bpc, 1], f32, kind="ExternalInput")
    out_d = nc.dram_tensor("scores", [bpc, n_c], f32, kind="ExternalOutput")
    if DEBUG:
        nquad_ = bpc // QUAD
        dbg_ex_d = nc.dram_tensor("dbg_ex", [nquad_, 128, n_c], f32, kind="ExternalOutput")
        dbg_eT_d = nc.dram_tensor("dbg_eT", [nquad_, 128, nt, 128], f32, kind="ExternalOutput")
        dbg_ctx_d = nc.dram_tensor("dbg_ctx", [nquad_, 128, D], f32, kind="ExternalOutput")
        dbg_oT_d = nc.dram_tensor("dbg_oT", [128, ND, nquad_, QUAD], f32, kind="ExternalOutput")
        dbg_ph_d = nc.dram_tensor("dbg_ph", [nquad_, 128, ND, QUAD], f32, kind="ExternalOutput")
        dbg_x_d = nc.dram_tensor("dbg_x", [bpc, n_c], f32, kind="ExternalOutput")
        dbg_s_d = nc.dram_tensor("dbg_s", [128, 2], f32, kind="ExternalOutput")

    def w_ap(d):  # [512,512] dram -> [128, 4, 512]
        return d[0:D, :].rearrange("(c p) d -> p c d", p=128)

    with tile.TileContext(nc) as tc:
        with tc.tile_pool(name="const", bufs=1) as constp, \
             tc.tile_pool(name="wmain", bufs=1) as wmain:
            ident = constp.tile([128, 128], f32, tag="ident")
            make_identity(nc, ident[:])
            identb = constp.tile([128, 128], bf16, tag="identb")
            nc.vector.tensor_copy(identb[:], ident[:])

            # persistent
            wv_t = wmain.tile([128, ND, D], bf16, tag="wv")
            gt_t = wmain.tile([128, ND, D], bf16, tag="gt")
            qhatT = wmain.tile([128, ND, bpc, H], bf16, tag="qhatT")
            ctxT_g = wmain.tile([128, ND, nquad, 32], bf16, tag="ctxTg")
            oT_g = wmain.tile([128, ND, nquad, QUAD], bf16, tag="oTg")
            x_all = wmain.tile([bpc, n_c], f32, tag="xall")
            npad32_t = wmain.tile([bpc, 1], f32, tag="npad32")
            nc.gpsimd.dma_start(wv_t[:], w_ap(wv_d))
            nc.gpsimd.dma_start(npad32_t[:], npad32_d[:])

            # ---------- phase 0 (f32r): qhatT, gt, state-derived queries ----
            with tc.tile_pool(name="w0", bufs=1) as w0, \
                 tc.tile_pool(name="ps0", bufs=2, space="PSUM") as ps0:
                wfc_t = w0.tile([128, ND, D], pdt, tag="wfc")
                wfc1_t = w0.tile([128, ND, D], pdt, tag="wfc1")
                wq_t = w0.tile([128, ND, D], pdt, tag="wq")
                wkT_t = w0.tile([128, ND, D], pdt, tag="wkT")
                woT_t = w0.tile([128, ND, D], pdt, tag="woT")
                wpT_t = w0.tile([128, ND, D], pdt, tag="wpT")
                wlast_t = w0.tile([128, ND], f32, tag="wlast")
                e0T_t = w0.tile([128, ND, bpc], pdt, tag="e0T")
                poolT_t = w0.tile([128, ND, bpc], pdt, tag="poolT")
                dcrep_t = w0.tile([128, bpc], f32, tag="dcrep")
                stateT = w0.tile([128, ND, bpc], pdt, tag="stateT")
                qT_t = w0.tile([128, ND, bpc], pdt, tag="qT")
                nc.gpsimd.dma_start(wfc_t[:], w_ap(wfc_d))
                nc.gpsimd.dma_start(wfc1_t[:], w_ap(wfc1_d))
                nc.gpsimd.dma_start(wq_t[:], w_ap(wq_d))
                nc.gpsimd.dma_start(wkT_t[:], w_ap(wkT_d))
                nc.gpsimd.dma_start(woT_t[:], w_ap(woT_d))
                nc.gpsimd.dma_start(wpT_t[:], w_ap(wpT_d))
                nc.gpsimd.dma_start(wlast_t[:], wlastT_d[:])
                nc.gpsimd.dma_start(e0T_t[:], e0T_d[:].rearrange("(c p) b -> p c b", p=128))
                nc.gpsimd.dma_start(poolT_t[:], poolT_d[:].rearrange("(c p) b -> p c b", p=128))
                nc.gpsimd.dma_start(dcrep_t[:], dcrep_d[:])

                # G^T = WoT.T @ WpT, scaled by norm_ptr
                for c in range(ND):
                    g_ps = ps0.tile([128, D], f32, tag="g_ps")
                    for kc in range(ND):
                        nc.tensor.matmul(g_ps[:], woT_t[:, kc, 128 * c:128 * (c + 1)],
                                         wpT_t[:, kc, :], start=(kc == 0), stop=(kc == ND - 1))
                    nc.scalar.mul(gt_t[:, c, :], g_ps[:], NORM_PTR)

                # stateT
                for c in range(ND):
                    st_ps = ps0.tile([128, bpc], f32, tag="st_ps")
                    for kc in range(ND):
                        nc.tensor.matmul(st_ps[:], wfc_t[:, kc, 128 * c:128 * (c + 1)],
                                         e0T_t[:, kc, :], start=(kc == 0), stop=False)
                    for kc in range(ND):
                        nc.tensor.matmul(st_ps[:], wfc1_t[:, kc, 128 * c:128 * (c + 1)],
                                         poolT_t[:, kc, :], start=False, stop=(kc == ND - 1))
                    nc.vector.scalar_tensor_tensor(
                        stateT[:, c, :], dcrep_t[:], wlast_t[:, c:c + 1], st_ps[:],
                        op0=ALU.mult, op1=ALU.add)

                # QT (scaled by norm_mha)
                for c in range(ND):
                    q_ps = ps0.tile([128, bpc], f32, tag="q_ps")
                    for kc in range(ND):
                        nc.tensor.matmul(q_ps[:], wq_t[:, kc, 128 * c:128 * (c + 1)],
                                         stateT[:, kc, :], start=(kc == 0), stop=(kc == ND - 1))
                    nc.scalar.mul(qT_t[:, c, :], q_ps[:], NORM_MHA)

                # qhatT[d, c, b, h] = (Wk_h^T q_b)_d  (64-row head blocks packed)
                for h in range(H):
                    pb = 64 * (h % 2)
                    for c in range(ND):
                        qq = ps0.tile([128, bpc], f32, tag="qq")
                        nc.tensor.matmul(
                            qq[:], wkT_t[pb:pb + 64, h // 2, 128 * c:128 * (c + 1)],
                            qT_t[pb:pb + 64, h // 2, :], start=True, stop=True)
                        nc.vector.tensor_copy(qhatT[:, c, :, h], qq[:])

            # ---------- main loop: quads of 4 batches ----------
            with tc.tile_pool(name="epool", bufs=3 * QUAD) as epool, \
                 tc.tile_pool(name="etpool", bufs=4 * QUAD) as etpool, \
                 tc.tile_pool(name="expool", bufs=3) as expool, \
                 tc.tile_pool(name="smpool", bufs=2) as smpool, \
                 tc.tile_pool(name="small", bufs=4) as smallp, \
                 tc.tile_pool(name="xsb", bufs=2) as xsbp, \
                 tc.tile_pool(name="cps", bufs=2, space="PSUM") as cps, \
                 tc.tile_pool(name="ctxps", bufs=1, space="PSUM") as ctxps, \
                 tc.tile_pool(name="tpsA", bufs=1, space="PSUM") as tpsA, \
                 tc.tile_pool(name="tpsB", bufs=1, space="PSUM") as tpsB, \
                 tc.tile_pool(name="otps", bufs=1, space="PSUM") as otps:
                st = {}

                def emit_dma(q):
                    ets, es = [], []
                    for j in range(QUAD):
                        b = QUAD * q + j
                        et_t = etpool.tile([128, ND, n_c], bf16, tag="ET")
                        nc.sync.dma_start(
                            et_t[:], EcT_d[b].rearrange("(c p) n -> p c n", p=128))
                        ets.append(et_t)
                        e_t = epool.tile([128, nt, D], bf16, tag="E")
                        nc.sync.dma_start(
                            e_t[:], Ec_d[b].rearrange("(t p) d -> p t d", p=128))
                        es.append(e_t)
                    npad_t = smallp.tile([128, 1], f32, tag="npad")
                    nc.sync.dma_start(npad_t[:], npadq_d[q])
                    st[q] = dict(et=ets, e=es, npad=npad_t)

                def emit_memset(q):
                    ex4b = expool.tile([128, n_c], bf16, tag="ex4b")
                    nc.gpsimd.memset(ex4b[:], 0.0)
                    st[q]['ex'] = ex4b

                def emit_compat(q):
                    # [128, 2, 512] so each half's accumulation region is
                    # bank-aligned (a matmul output must not cross a PSUM bank)
                    cp = cps.tile([128, 2, 512], f32, tag="cp")
                    ets = st[q]['et']
                    for half in range(2):
                        for j in range(QUAD):
                            for c in range(ND):
                                nc.tensor.matmul(
                                    cp[32 * j:32 * j + 8, half, 0:nh],
                                    qhatT[:, c, QUAD * q + j, :],
                                    ets[j][:, c, half * nh:(half + 1) * nh],
                                    start=(c == 0), stop=(c == ND - 1),
                                    tile_position=(0, 32 * j))
                    st[q]['cp'] = cp

                def emit_exp(q):
                    cp, ex4b = st[q]['cp'], st[q]['ex']
                    s_t = smallp.tile([128, 1], f32, tag="s")
                    for j in range(QUAD):
                        nc.scalar.activation(
                            ex4b[32 * j:32 * j + 8, :], cp[32 * j:32 * j + 8, :, 0:nh],
                            AF.Exp, bias=0.0, scale=1.0,
                            accum_out=s_t[32 * j:32 * j + 8, :])
                    st[q]['s'] = s_t

                def emit_expT(q):
                    ex4b, s_t, npad_t = st[q]['ex'], st[q]['s'], st[q]['npad']
                    # softmax denominators: 1 / (sum - npad)
                    r_t = smallp.tile([128, 1], f32, tag="r")
                    nc.vector.tensor_sub(r_t[:], s_t[:], npad_t[:])
                    nc.vector.reciprocal(r_t[:], r_t[:])
                    st[q]['r'] = r_t
                    tpT = tpsA.tile([128, nt, 128], bf16, tag="tpT")
                    for t in range(nt):
                        nc.tensor.transpose(
                            tpT[:, t, :], ex4b[:, 128 * t:128 * (t + 1)], identb[:])
                    expT4b = smpool.tile([128, nt, 128], bf16, tag="expT")
                    nc.vector.tensor_copy(expT4b[:], tpT[:])
                    st[q]['expT'] = expT4b

                def emit_ctx(q):
                    expT4b, es = st[q]['expT'], st[q]['e']
                    ctxp = ctxps.tile([128, D], f32, tag="ctxp")
                    nc.vector.memset(ctxp[:], 0.0)
                    for j in range(QUAD):
                        for t in range(nt):
                            nc.tensor.matmul(
                                ctxp[32 * j:32 * j + 8, :],
                                expT4b[:, t, 32 * j:32 * j + 8],
                                es[j][:, t, :],
                                start=(t == 0), stop=(t == nt - 1),
                                tile_position=(0, 32 * j))
                    st[q]['ctxp'] = ctxp

                def emit_ctxcopy(q):
                    ctxp, r_t = st[q]['ctxp'], st[q]['r']
                    ctx4b = smpool.tile([128, D], bf16, tag="ctx4b")
                    nc.scalar.activation(ctx4b[:], ctxp[:], AF.Copy,
                                         bias=0.0, scale=r_t[:, 0:1])
                    st[q]['ctx'] = ctx4b

                def emit_ctxT(q):
                    ctx4b = st[q]['ctx']
                    tpC = tpsB.tile([128, ND, 128], bf16, tag="tpC")
                    for c in range(ND):
                        nc.tensor.transpose(
                            tpC[:, c, :], ctx4b[:, 128 * c:128 * (c + 1)], identb[:])
                    # gather valid cols m=32j+h -> ctxT_g[:, c, q, 8j+h]
                    nc.vector.tensor_copy(
                        ctxT_g[:, :, q, :].rearrange("p c (j h) -> p c j h", j=QUAD),
                        tpC[:, :, :].rearrange("p c (j x) -> p c j x", j=QUAD)[:, :, :, 0:8])

                def emit_o(q):
                    # oT[64h+k, j] = sum_d ctx[j,h,d] Wv[d, 64h+k]
                    oTp = otps.tile([128, ND, QUAD], f32, tag="op")
                    rhs = ctxT_g[:, :, q, :].rearrange("p c (j h) -> p c j h", h=8)
                    for cc in range(ND):
                        for h in (2 * cc, 2 * cc + 1):
                            pb = 64 * (h % 2)
                            for c in range(ND):
                                nc.tensor.matmul(
                                    oTp[pb:pb + 64, cc, :],
                                    wv_t[:, c, 64 * h:64 * (h + 1)],
                                    rhs[:, c, :, h],
                                    start=(c == 0), stop=(c == ND - 1),
                                    tile_position=(0, pb))
                    nc.vector.tensor_copy(oT_g[:, :, q, :], oTp[:])

                def emit_phat(q):
                    php = otps.tile([128, ND, QUAD], f32, tag="op")
                    for c2 in range(ND):
                        for c in range(ND):
                            nc.tensor.matmul(
                                php[:, c2, :],
                                gt_t[:, c, 128 * c2:128 * (c2 + 1)],
                                oT_g[:, c, q, :],
                                start=(c == 0), stop=(c == ND - 1))
                    phatT_q = smallp.tile([128, ND, QUAD], bf16, tag="phatT")
                    nc.vector.tensor_copy(phatT_q[:], php[:])
                    st[q]['phat'] = phatT_q

                def emit_c2(q):
                    phatT_q, ets = st[q]['phat'], st[q]['et']
                    cp2 = cps.tile([128, 2, 512], f32, tag="cp")
                    for half in range(2):
                        for j in range(QUAD):
                            for c in range(ND):
                                nc.tensor.matmul(
                                    cp2[32 * j:32 * j + 1, half, 0:nh],
                                    phatT_q[:, c, j:j + 1],
                                    ets[j][:, c, half * nh:(half + 1) * nh],
                                    start=(c == 0), stop=(c == ND - 1),
                                    tile_position=(0, 32 * j))
                    x_sb = xsbp.tile([128, 2, nh], f32, tag="xsb")
                    for j in range(QUAD):
                        eng = nc.scalar.copy if j % 2 == 0 else nc.vector.tensor_copy
                        eng(x_sb[32 * j:32 * j + 1, :, :], cp2[32 * j:32 * j + 1, :, 0:nh])
                        nc.gpsimd.dma_start(
                            x_all[QUAD * q + j:QUAD * q + j + 1, :],
                            x_sb[32 * j:32 * j + 1, :, :].rearrange("p a b -> p (a b)"))
                    del st[q]

                def emit_debug(q):
                    if not DEBUG:
                        return
                    if q == 0:
                        d5 = wmain.tile([128, 2], f32, tag="d5")
                        nc.vector.tensor_copy(d5[:, 0:1], st[q]['s'][:])
                        nc.vector.tensor_copy(d5[:, 1:2], st[q]['r'][:])
                        nc.sync.dma_start(dbg_s_d[:], d5[:])
                    d1 = wmain.tile([128, n_c], f32, tag="d1")
                    nc.scalar.copy(d1[:], st[q]['ex'][:])
                    nc.sync.dma_start(dbg_ex_d[q], d1[:])
                    d6 = wmain.tile([128, nt, 128], f32, tag="d6")
                    nc.scalar.copy(d6[:], st[q]['expT'][:])
                    nc.sync.dma_start(dbg_eT_d[q], d6[:])
                    d2 = wmain.tile([128, D], f32, tag="d2")
                    nc.scalar.copy(d2[:], st[q]['ctx'][:])
                    nc.sync.dma_start(dbg_ctx_d[q], d2[:])
                    d4 = wmain.tile([128, ND, QUAD], f32, tag="d4")
                    nc.scalar.copy(d4[:], st[q]['phat'][:])
                    nc.sync.dma_start(dbg_ph_d[q], d4[:])

                def emit_ptr():
                    if DEBUG:
                        d3 = wmain.tile([128, ND, nquad, QUAD], f32, tag="d3")
                        nc.vector.tensor_copy(d3[:], oT_g[:])
                        nc.sync.dma_start(dbg_oT_d[:], d3[:])
                        nc.sync.dma_start(dbg_x_d[:], x_all[:])
                    # x_all holds compat2 for all 32 batches; softmax(10*tanh(x))
                    th = wmain.tile([bpc, n_c], f32, tag="th")
                    nc.scalar.activation(th[:], x_all[:], AF.Tanh)
                    e2 = wmain.tile([bpc, n_c], f32, tag="e2")
                    s2 = wmain.tile([bpc, 1], f32, tag="s2")
                    nc.scalar.activation(e2[:], th[:], AF.Exp,
                                         bias=0.0, scale=10.0, accum_out=s2[:])
                    r2 = wmain.tile([bpc, 1], f32, tag="r2")
                    nc.vector.tensor_sub(r2[:], s2[:], npad32_t[:])
                    nc.vector.reciprocal(r2[:], r2[:])
                    sc = wmain.tile([bpc, n_c], f32, tag="sc")
                    nc.vector.tensor_scalar_mul(sc[:], e2[:], r2[:])
                    nc.sync.dma_start(out_d[:], sc[:])

                nquad_r = nquad
                emit_dma(0)
                for r in range(nquad_r + 2):
                    if r + 1 < nquad_r:
                        emit_dma(r + 1)
                    if r < nquad_r:
                        emit_memset(r)
                    if 1 <= r <= nquad_r:
                        emit_expT(r - 1)
                    if r < nquad_r:
                        emit_compat(r)
                        emit_exp(r)
                    if 1 <= r <= nquad_r:
                        emit_ctx(r - 1)
                        emit_ctxcopy(r - 1)
                    if 2 <= r <= nquad_r + 1:
                        emit_ctxT(r - 2)
                        emit_o(r - 2)
                        emit_phat(r - 2)
                        emit_debug(r - 2)
                        emit_c2(r - 2)
                emit_ptr()

    nc.finalize()
    return nc


def _host_prep(inputs, n_c=None):
    E = np.ascontiguousarray(inputs['encoder_inputs'], dtype=np.float32)
    mask = np.asarray(inputs['mask'])
    unm = (mask == 0)
    counts = unm.sum(axis=1).astype(np.int64)
    if n_c is None:
        n_c = max(512, int(np.ceil(counts.max() / 128) * 128))
    idx = np.zeros((B, n_c), dtype=np.int64)
    for b in range(B):
        ii = np.nonzero(unm[b])[0]
        k = min(len(ii), n_c)
        idx[b, :k] = ii[:k]
    Ec = np.take_along_axis(E, idx[:, :, None], axis=1)   # [B, n_c, D]
    # zero the padding rows so padded compat entries are exactly 0
    pad = np.arange(n_c)[None, :] >= counts[:, None]      # [B, n_c]
    Ec[pad] = 0.0
    return Ec, idx, counts, n_c


def _in_maps(inputs, Ec, counts, n_c, bpc=BPC):
    import ml_dtypes
    bf16 = ml_dtypes.bfloat16
    nquad = bpc // QUAD
    W_fc = np.asarray(inputs['W_fc'], dtype=np.float32)
    wlastT = np.ascontiguousarray(W_fc[D].reshape(ND, 128).T)        # [128, 4]
    wkT = np.ascontiguousarray(np.asarray(inputs['Wk_mha']).T)
    woT = np.ascontiguousarray(np.asarray(inputs['Wo']).T)
    wpT = np.ascontiguousarray(np.asarray(inputs['Wk_ptr']).T)
    pool = np.asarray(inputs['pool'], dtype=np.float32)
    dc = np.asarray(inputs['dynamic_capacity'], dtype=np.float32)
    Ecb = Ec.astype(bf16)
    npad = (n_c - counts).astype(np.float32)
    maps = []
    for i in range(NCORES):
        b0 = i * bpc
        npadq = np.repeat(npad[b0:b0 + bpc].reshape(nquad, QUAD), 32, axis=1)
        m = {
            "Ec": np.ascontiguousarray(Ecb[b0:b0 + bpc]),
            "EcT": np.ascontiguousarray(Ecb[b0:b0 + bpc].transpose(0, 2, 1)),
            "W_fc": W_fc,
            "W_fc1": np.asarray(inputs['W_fc1'], dtype=np.float32),
            "Wq": np.asarray(inputs['Wq'], dtype=np.float32),
            "WkT": wkT,
            "Wv": np.asarray(inputs['Wv'], dtype=np.float32).astype(bf16),
            "WoT": woT,
            "WpT": wpT,
            "wlastT": wlastT,
            "E0T": np.ascontiguousarray(Ec[b0:b0 + bpc, 0, :].T),
            "poolT": np.ascontiguousarray(pool[b0:b0 + bpc].T),
            "dcrep": np.ascontiguousarray(np.broadcast_to(dc[b0:b0 + bpc, 0], (128, bpc))),
            "npadq": np.ascontiguousarray(npadq.reshape(nquad, 128, 1)),
            "npad32": np.ascontiguousarray(npad[b0:b0 + bpc].reshape(bpc, 1)),
        }
        maps.append(m)
    return maps


_cache = {}


def _get_nc(n_c, bpc):
    key = (n_c, bpc)
    if key not in _cache:
        _cache[key] = _build(n_c, bpc)
    return _cache[key]


def run(inputs, trace=False, **_ignored):
    from concourse.bass_utils import run_bass_kernel_spmd
    Ec, idx, counts, n_c = _host_prep(inputs)
    nc = _get_nc(n_c, BPC)
    maps = _in_maps(inputs, Ec, counts, n_c, BPC)
    res = run_bass_kernel_spmd(nc, maps, list(range(NCORES)), trace=trace)
    scores = np.zeros((B, N), dtype=np.float32)
    for i in range(NCORES):
        sc = res.results[i]["scores"]
        for j in range(BPC):
            b = i * BPC + j
            c = counts[b]
            scores[b, idx[b, :c]] = sc[j, :c]
    return scores, res


def kernel(**inputs) -> np.ndarray:
    scores, _ = run(inputs, trace=False)
    return scores
